# revision 1
# baseline (speedup 1.0000x reference)
"""Trainium2 Bass kernel for Bahdanau-style additive attention (nn_Attention).

reference math (per batch b, all fp32):
  q_attn = query @ Wq_w + Wq_b                       [B,Tq,U]
  k_attn = value @ Wk_w + Wk_b                       [B,Tv,U]
  scores[b,q,v] = sum_u V_w[u]*tanh(q_attn[b,q,u]+k_attn[b,v,u]) + V_b
  scores -= 1e9 * (~v_mask)
  weights = softmax(scores, axis=-1)
  attn = weights @ value
  result = layer_norm(query + attn) * gamma + beta
  returns (result, weights)

Sharding: data-parallel over batch B=8 -> one batch element per NeuronCore.

Device-side layout strategy (per core):
  * u (=UNITS=128) lives on SBUF partitions for the feats phase.
  * S[u, q*Tv+v] = q_attnT[u,q] + k_attnT[u,v] built by DVE tensor_scalar
    (per-q per-partition scalar, 2x fp32 mode).
  * tanh on ACT in big chunks, output fp16 (feats magnitudes <= 1).
  * scoresT[v,q] via per-q matmuls: lhsT = feats[u, v-half] (fp16 weights),
    rhs = V_w [u,1] -> psum column [v-half, q]. M=128 on v partitions.
  * softmax in transposed space: exp with mask as per-partition bias;
    denominator via ones-matmul (partition reduce); reciprocal; broadcast
    back with a rank-1 matmul; weightsT stays as lhsT for the attention
    matmul.  V_b is softmax-shift-invariant and droppe.
  * weights are PE-transposed back to [q,v] for the DRAM output.
  * residual + layernorm with DVE reductions; sqrt on ACT refined by one
    Newton step (ACT sqrt table has a loose ULP budget).
No per-row max subtraction before exp: |scores| <= sum|V_w| (~9 for unit
normal weights), far inside fp32 exp range; masked scores are -1e9 ->
exp underflows to exactly 0 like the reference.
"""

import numpy as np

B, TQ, TV, D, U = 8, 128, 256, 256, 128
LN_EPS = 1e-3
N_CORES = 8

_CACHE = {}


def _build_program(stage=4, repeat=0):
    from contextlib import ExitStack
    import concourse.bacc as bacc
    import concourse.tile as tile
    from concourse import mybir
    import concourse.bass as bass

    f32 = mybir.dt.float32
    f16 = mybir.dt.float16
    AF = mybir.ActivationFunctionType
    ALU = mybir.AluOpType
    AX = mybir.AxisListType

    nc = bacc.Bacc("TRN2", target_bir_lowering=False, debug=False)

    def din(name, shape):
        return nc.dram_tensor(name, shape, f32, kind="ExternalInput").ap()

    qT = din("qT", [D, TQ])          # query transposed (d, q)
    qn = din("qn", [TQ, D])          # query natural (for residual)
    vT = din("vT", [D, TV])          # value transposed (d, v)
    vn = din("vn", [TV, D])          # value natural (v, d)
    wq = din("wq", [D, U])
    wk = din("wk", [D, U])
    wqb = din("wqb", [U, 1])
    wkb = din("wkb", [U, 1])
    vw = din("vw", [U, 1])
    maskc = din("maskc", [128, TV // 128])  # -1e9*(~mask), v-in-half on rows
    gam = din("gam", [TQ, D])        # gamma replicated along q
    bet = din("bet", [TQ, D])        # beta replicated along q
    iden = din("iden", [128, 128])   # identity for PE transpose
    ones_c = din("ones_c", [128, 1])  # ones column (denominator lhsT)
    ones_r = din("ones_r", [1, 128])  # ones row (broadcast lhsT)

    out_res = nc.dram_tensor("out_res", [TQ, D], f32, kind="ExternalOutput").ap()
    out_w = nc.dram_tensor("out_w", [TQ, TV], f32, kind="ExternalOutput").ap()

    NVH = TV // 128  # number of 128-wide v halves

    with tile.TileContext(nc) as tc, ExitStack() as ctx:
        const = ctx.enter_context(tc.tile_pool(name="const", bufs=1))
        work = ctx.enter_context(tc.tile_pool(name="work", bufs=2))
        spool = ctx.enter_context(tc.tile_pool(name="spool", bufs=3))
        fpool = ctx.enter_context(tc.tile_pool(name="fpool", bufs=3))
        psum = ctx.enter_context(tc.tile_pool(name="psum", bufs=1, space="PSUM"))
        psc = ctx.enter_context(tc.tile_pool(name="psc", bufs=1, space="PSUM"))

        def body():
            def load(ap, shape, name, pool=const):
                t = pool.tile(shape, f32, name=name)
                nc.sync.dma_start(out=t[:, :], in_=ap)
                return t

            # ---- load constants / operands --------------------------------
            qT_sb = [load(qT[i * 128:(i + 1) * 128, :], [128, TQ], name=f"qT_sb{i}")
                     for i in range(2)]
            vT_sb = [load(vT[i * 128:(i + 1) * 128, :], [128, TV], name=f"vT_sb{i}")
                     for i in range(2)]
            vn_sb = [load(vn[i * 128:(i + 1) * 128, :], [128, D], name=f"vn_sb{i}")
                     for i in range(NVH)]
            wq_sb = [load(wq[i * 128:(i + 1) * 128, :], [128, U], name=f"wq_sb{i}")
                     for i in range(2)]
            wk_sb = [load(wk[i * 128:(i + 1) * 128, :], [128, U], name=f"wk_sb{i}")
                     for i in range(2)]
            qn_sb = load(qn, [TQ, D], "qn_sb")
            wqb_sb = load(wqb, [U, 1], "wqb_sb")
            wkb_sb = load(wkb, [U, 1], "wkb_sb")
            maskc_sb = load(maskc, [128, NVH], "maskc_sb")
            gam_sb = load(gam, [TQ, D], "gam_sb")
            bet_sb = load(bet, [TQ, D], "bet_sb")
            iden_sb = load(iden, [128, 128], "iden_sb")
            ones_c_sb = load(ones_c, [128, 1], "ones_c_sb")
            ones_r_sb = load(ones_r, [1, 128], "ones_r_sb")
            vw_sb = load(vw, [U, 1], "vw_sb")
            # V_w as fp16 for the feats matmuls
            vw16_sb = const.tile([U, 1], f16)
            nc.vector.tensor_copy(vw16_sb[:, :], vw_sb[:, :])

            def finish_dbg(a, b):
                """debug outputs: a,b are [128, TQ] and [128, <=TV] tiles"""
                w_dbg = work.tile([TQ, TV], f32, name="w_dbg")
                nc.gpsimd.memset(w_dbg[:, :], 0.0)
                nc.vector.tensor_copy(w_dbg[:, 0:a.shape[1]], a[:, :])
                nc.sync.dma_start(out=out_w, in_=w_dbg[:, :])
                r_dbg = work.tile([TQ, D], f32, name="r_dbg")
                nc.gpsimd.memset(r_dbg[:, :], 0.0)
                nc.vector.tensor_copy(r_dbg[:, 0:b.shape[1]], b[:, :])
                nc.sync.dma_start(out=out_res, in_=r_dbg[:, :])

            # ---- q_attnT [u, q], k_attnT [u, v] ---------------------------
            ps_qa = psum.tile([U, TQ], f32, tag="pa")
            nc.tensor.matmul(ps_qa[:, :], wq_sb[0][:, :], qT_sb[0][:, :],
                             start=True, stop=False)
            nc.tensor.matmul(ps_qa[:, :], wq_sb[1][:, :], qT_sb[1][:, :],
                             start=False, stop=True)
            qa_sb = work.tile([U, TQ], f32)
            nc.scalar.add(qa_sb[:, :], ps_qa[:, :], wqb_sb[:, 0:1])

            ps_ka = psum.tile([U, TV], f32, tag="pa")
            nc.tensor.matmul(ps_ka[:, :], wk_sb[0][:, :], vT_sb[0][:, :],
                             start=True, stop=False)
            nc.tensor.matmul(ps_ka[:, :], wk_sb[1][:, :], vT_sb[1][:, :],
                             start=False, stop=True)
            ka_sb = work.tile([U, TV], f32)
            nc.scalar.add(ka_sb[:, :], ps_ka[:, :], wkb_sb[:, 0:1])

            if stage == 1:
                finish_dbg(qa_sb, ka_sb)

            if stage >= 2:
                # ---- feats + scoresT --------------------------------------
                ps_scT = [psc.tile([128, TQ], f32, tag=f"scT{h}", name=f"ps_scT{h}")
                          for h in range(NVH)]
                QCHUNK = 16
                for q0 in range(0, TQ, QCHUNK):
                    s_ch = spool.tile([U, QCHUNK * TV], f32, tag="s")
                    for j in range(QCHUNK):
                        nc.vector.tensor_scalar_add(
                            s_ch[:, j * TV:(j + 1) * TV], ka_sb[:, :],
                            qa_sb[:, q0 + j:q0 + j + 1])
                    f_ch = fpool.tile([U, QCHUNK * TV], f16, tag="f")
                    nc.scalar.activation(f_ch[:, :], s_ch[:, :], AF.Tanh)
                    for j in range(QCHUNK):
                        q = q0 + j
                        for h in range(NVH):
                            nc.tensor.matmul(
                                ps_scT[h][:, q:q + 1],
                                f_ch[:, j * TV + h * 128: j * TV + (h + 1) * 128],
                                vw16_sb[:, 0:1],
                                start=True, stop=True)
                if stage == 2:
                    t0 = work.tile([128, TQ], f32, name="t0")
                    nc.vector.tensor_copy(t0[:, :], ps_scT[0][:, :])
                    t1 = work.tile([128, TQ], f32, name="t1")
                    nc.vector.tensor_copy(t1[:, :], ps_scT[1][:, :])
                    cat = work.tile([128, 2 * TQ], f32, name="cat")
                    nc.vector.tensor_copy(cat[:, 0:TQ], t0[:, :])
                    nc.vector.tensor_copy(cat[:, TQ:2 * TQ], t1[:, :])
                    finish_dbg(cat, qa_sb)

            if stage >= 3:
                # ---- softmax in transposed space --------------------------
                expT_sb = []
                for h in range(NVH):
                    e = work.tile([128, TQ], f32, tag=f"expT{h}", name=f"expT{h}")
                    nc.scalar.activation(e[:, :], ps_scT[h][:, :], AF.Exp,
                                         bias=maskc_sb[:, h:h + 1])
                    expT_sb.append(e)

                ps_den = psum.tile([1, TQ], f32, tag="pb")
                for h in range(NVH):
                    nc.tensor.matmul(ps_den[:, :], ones_c_sb[:, :], expT_sb[h][:, :],
                                     start=(h == 0), stop=(h == NVH - 1))
                den_sb = work.tile([1, TQ], f32)
                nc.vector.tensor_copy(den_sb[:, :], ps_den[:, :])
                rinv_sb = work.tile([1, TQ], f32)
                nc.vector.reciprocal(rinv_sb[:, :], den_sb[:, :])
                ps_rrep = psum.tile([128, TQ], f32, tag="pb")
                nc.tensor.matmul(ps_rrep[:, :], ones_r_sb[:, :], rinv_sb[:, :],
                                 start=True, stop=True)
                rrep_sb = work.tile([128, TQ], f32)
                nc.vector.tensor_copy(rrep_sb[:, :], ps_rrep[:, :])

                wT_sb = []
                for h in range(NVH):
                    w = work.tile([128, TQ], f32, tag=f"wT{h}", name=f"wT{h}")
                    nc.vector.tensor_mul(w[:, :], expT_sb[h][:, :], rrep_sb[:, :])
                    wT_sb.append(w)

                if stage == 3:
                    cat = work.tile([128, 2 * TQ], f32, name="cat")
                    nc.vector.tensor_copy(cat[:, 0:TQ], wT_sb[0][:, :])
                    nc.vector.tensor_copy(cat[:, TQ:2 * TQ], wT_sb[1][:, :])
                    finish_dbg(cat, rrep_sb)

            if stage >= 4:
                # ---- weights natural [q, v] for output --------------------
                w_sb = work.tile([TQ, TV], f32)
                for h in range(NVH):
                    ps_wn = psum.tile([128, 128], f32, tag="pb", name="ps_wn")
                    nc.tensor.transpose(ps_wn[:, :], wT_sb[h][:, :], iden_sb[:, :])
                    nc.vector.tensor_copy(w_sb[:, h * 128:(h + 1) * 128], ps_wn[:, :])
                nc.sync.dma_start(out=out_w, in_=w_sb[:, :])

                # ---- attention output + residual + layernorm --------------
                ps_at = psum.tile([TQ, D], f32, tag="pc")
                for h in range(NVH):
                    nc.tensor.matmul(ps_at[:, :], wT_sb[h][:, :], vn_sb[h][:, :],
                                     start=(h == 0), stop=(h == NVH - 1))

                x_sb = work.tile([TQ, D], f32)
                nc.vector.tensor_add(x_sb[:, :], qn_sb[:, :], ps_at[:, :])

                if stage == 35:
                    nc.sync.dma_start(out=out_res, in_=x_sb[:, :])

            if stage >= 4 and stage != 35:
                ssum = work.tile([TQ, 1], f32)
                nc.vector.reduce_sum(ssum[:, :], x_sb[:, :], axis=AX.X)
                negmu = work.tile([TQ, 1], f32)
                nc.vector.tensor_scalar_mul(negmu[:, :], ssum[:, :], -1.0 / D)
                xc_sb = work.tile([TQ, D], f32)
                nc.vector.tensor_scalar_add(xc_sb[:, :], x_sb[:, :], negmu[:, 0:1])

                xsq = work.tile([TQ, D], f32)
                nc.vector.tensor_mul(xsq[:, :], xc_sb[:, :], xc_sb[:, :])
                vsum = work.tile([TQ, 1], f32)
                nc.vector.reduce_sum(vsum[:, :], xsq[:, :], axis=AX.X)
                veps = work.tile([TQ, 1], f32)
                nc.vector.tensor_scalar(veps[:, :], vsum[:, :], 1.0 / D, LN_EPS,
                                        op0=ALU.mult, op1=ALU.add)
                # rstd = 1/sqrt(veps) via Newton iteration on DVE only
                # (ACT sqrt and tensor_tensor_reduce both crash this runtime).
                u_t = work.tile([TQ, 1], f32)
                nc.vector.tensor_scalar_add(u_t[:, :], veps[:, :], 1.0)
                w_t = work.tile([TQ, 1], f32)
                nc.vector.reciprocal(w_t[:, :], u_t[:, :])
                y_t = work.tile([TQ, 1], f32, bufs=8)
                nc.vector.tensor_scalar_mul(y_t[:, :], w_t[:, :], 2.0)
                e_t = work.tile([TQ, 1], f32)
                nc.vector.tensor_scalar_mul(e_t[:, :], veps[:, :], 0.5)
                for it in range(6):
                    a_t = work.tile([TQ, 1], f32, tag="nwa", name=f"nwa{it}")
                    nc.vector.tensor_mul(a_t[:, :], y_t[:, :], y_t[:, :])
                    b_t = work.tile([TQ, 1], f32, tag="nwb", name=f"nwb{it}")
                    nc.vector.tensor_mul(b_t[:, :], e_t[:, :], a_t[:, :])
                    c_t = work.tile([TQ, 1], f32, tag="nwc", name=f"nwc{it}")
                    nc.vector.tensor_scalar(c_t[:, :], b_t[:, :], -1.0, 1.5,
                                            op0=ALU.mult, op1=ALU.add)
                    y_new = work.tile([TQ, 1], f32, tag="nwy", name=f"nwy{it}")
                    nc.vector.tensor_mul(y_new[:, :], y_t[:, :], c_t[:, :])
                    y_t = y_new
                xn_sb = work.tile([TQ, D], f32)
                nc.vector.tensor_scalar_mul(xn_sb[:, :], xc_sb[:, :], y_t[:, 0:1])
                res_sb = work.tile([TQ, D], f32)
                nc.vector.tensor_mul(res_sb[:, :], xn_sb[:, :], gam_sb[:, :])
                nc.vector.tensor_add(res_sb[:, :], res_sb[:, :], bet_sb[:, :])
                nc.sync.dma_start(out=out_res, in_=res_sb[:, :])


        if repeat:
            with tc.For_i(0, repeat, 1, hint_engines=(
                    mybir.EngineType.PE, mybir.EngineType.DVE,
                    mybir.EngineType.Activation, mybir.EngineType.SP,
                    mybir.EngineType.Pool)):
                body()
        else:
            body()

    nc.compile()
    return nc


def _host_prep(query, value, v_mask, Wq_w, Wq_b, Wk_w, Wk_b, V_w, ln_gamma,
               ln_beta):
    """Build the per-core input maps."""
    in_maps = []
    iden = np.eye(128, dtype=np.float32)
    ones_c = np.ones((128, 1), np.float32)
    ones_r = np.ones((1, 128), np.float32)
    gam = np.broadcast_to(ln_gamma.astype(np.float32), (TQ, D)).copy()
    bet = np.broadcast_to(ln_beta.astype(np.float32), (TQ, D)).copy()
    wqb = Wq_b.astype(np.float32).reshape(U, 1)
    wkb = Wk_b.astype(np.float32).reshape(U, 1)
    vw = V_w.astype(np.float32).reshape(U, 1)
    for b in range(B):
        q = np.ascontiguousarray(query[b].astype(np.float32))
        v = np.ascontiguousarray(value[b].astype(np.float32))
        maskc = (-1e9 * (~v_mask[b]).astype(np.float32)).reshape(TV // 128, 128).T
        in_maps.append({
            "qT": np.ascontiguousarray(q.T),
            "qn": q,
            "vT": np.ascontiguousarray(v.T),
            "vn": v,
            "wq": np.ascontiguousarray(Wq_w.astype(np.float32)),
            "wk": np.ascontiguousarray(Wk_w.astype(np.float32)),
            "wqb": wqb, "wkb": wkb, "vw": vw,
            "maskc": np.ascontiguousarray(maskc),
            "gam": gam, "bet": bet, "iden": iden,
            "ones_c": ones_c, "ones_r": ones_r,
        })
    return in_maps


def kernel(query, value, v_mask, Wq_w, Wq_b, Wk_w, Wk_b, V_w, V_b, ln_gamma,
           ln_beta):
    from concourse.bass_utils import run_bass_kernel_spmd

    if "nc" not in _CACHE:
        _CACHE["nc"] = _build_program()
    nc = _CACHE["nc"]
    in_maps = _host_prep(query, value, v_mask, Wq_w, Wq_b, Wk_w, Wk_b, V_w,
                         ln_gamma, ln_beta)
    res = run_bass_kernel_spmd(nc, in_maps, core_ids=list(range(N_CORES)))
    result = np.stack([res.results[b]["out_res"] for b in range(B)])
    weights = np.stack([res.results[b]["out_w"] for b in range(B)])
    return result.astype(np.float32), weights.astype(np.float32)


def _build_phase_program(phase, repeat):
    """Isolated phase benchmark program: loop contains only one phase."""
    from contextlib import ExitStack
    import concourse.bacc as bacc
    import concourse.tile as tile
    from concourse import mybir

    f32 = mybir.dt.float32
    f16 = mybir.dt.float16
    AF = mybir.ActivationFunctionType
    ALU = mybir.AluOpType
    AX = mybir.AxisListType

    nc = bacc.Bacc("TRN2", target_bir_lowering=False, debug=False)

    def din(name, shape):
        return nc.dram_tensor(name, shape, f32, kind="ExternalInput").ap()

    qT = din("qT", [D, TQ]); qn = din("qn", [TQ, D])
    vT = din("vT", [D, TV]); vn = din("vn", [TV, D])
    wq = din("wq", [D, U]); wk = din("wk", [D, U])
    wqb = din("wqb", [U, 1]); wkb = din("wkb", [U, 1]); vw = din("vw", [U, 1])
    maskc = din("maskc", [128, TV // 128])
    gam = din("gam", [TQ, D]); bet = din("bet", [TQ, D])
    iden = din("iden", [128, 128])
    ones_c = din("ones_c", [128, 1]); ones_r = din("ones_r", [1, 128])
    out_res = nc.dram_tensor("out_res", [TQ, D], f32, kind="ExternalOutput").ap()
    out_w = nc.dram_tensor("out_w", [TQ, TV], f32, kind="ExternalOutput").ap()
    NVH = TV // 128

    with tile.TileContext(nc) as tc, ExitStack() as ctx:
        const = ctx.enter_context(tc.tile_pool(name="const", bufs=1))
        work = ctx.enter_context(tc.tile_pool(name="work", bufs=2))
        spool = ctx.enter_context(tc.tile_pool(name="spool", bufs=3))
        fpool = ctx.enter_context(tc.tile_pool(name="fpool", bufs=3))
        psum = ctx.enter_context(tc.tile_pool(name="psum", bufs=1, space="PSUM"))
        psc = ctx.enter_context(tc.tile_pool(name="psc", bufs=1, space="PSUM"))

        def load(ap, shape, name, pool=const):
            t = pool.tile(shape, f32, name=name)
            nc.sync.dma_start(out=t[:, :], in_=ap)
            return t

        qT_sb = [load(qT[i*128:(i+1)*128, :], [128, TQ], name=f"qT_sb{i}") for i in range(2)]
        vT_sb = [load(vT[i*128:(i+1)*128, :], [128, TV], name=f"vT_sb{i}") for i in range(2)]
        vn_sb = [load(vn[i*128:(i+1)*128, :], [128, D], name=f"vn_sb{i}") for i in range(NVH)]
        wq_sb = [load(wq[i*128:(i+1)*128, :], [128, U], name=f"wq_sb{i}") for i in range(2)]
        wk_sb = [load(wk[i*128:(i+1)*128, :], [128, U], name=f"wk_sb{i}") for i in range(2)]
        qn_sb = load(qn, [TQ, D], "qn_sb")
        wqb_sb = load(wqb, [U, 1], "wqb_sb")
        wkb_sb = load(wkb, [U, 1], "wkb_sb")
        maskc_sb = load(maskc, [128, NVH], "maskc_sb")
        gam_sb = load(gam, [TQ, D], "gam_sb")
        bet_sb = load(bet, [TQ, D], "bet_sb")
        iden_sb = load(iden, [128, 128], "iden_sb")
        ones_c_sb = load(ones_c, [128, 1], "ones_c_sb")
        ones_r_sb = load(ones_r, [1, 128], "ones_r_sb")
        vw_sb = load(vw, [U, 1], "vw_sb")
        vw16_sb = const.tile([U, 1], f16)
        nc.vector.tensor_copy(vw16_sb[:, :], vw_sb[:, :])

        ps_qa = psum.tile([U, TQ], f32, tag="pa")
        nc.tensor.matmul(ps_qa[:, :], wq_sb[0][:, :], qT_sb[0][:, :], start=True, stop=False)
        nc.tensor.matmul(ps_qa[:, :], wq_sb[1][:, :], qT_sb[1][:, :], start=False, stop=True)
        qa_sb = work.tile([U, TQ], f32)
        nc.scalar.add(qa_sb[:, :], ps_qa[:, :], wqb_sb[:, 0:1])
        ps_ka = psum.tile([U, TV], f32, tag="pa")
        nc.tensor.matmul(ps_ka[:, :], wk_sb[0][:, :], vT_sb[0][:, :], start=True, stop=False)
        nc.tensor.matmul(ps_ka[:, :], wk_sb[1][:, :], vT_sb[1][:, :], start=False, stop=True)
        ka_sb = work.tile([U, TV], f32)
        nc.scalar.add(ka_sb[:, :], ps_ka[:, :], wkb_sb[:, 0:1])

        QCHUNK = 16
        # prebuilt chunks for act/pe phases
        s_pre = const.tile([U, QCHUNK * TV], f32, name="s_pre")
        for j in range(QCHUNK):
            nc.vector.tensor_scalar_add(s_pre[:, j*TV:(j+1)*TV], ka_sb[:, :],
                                        qa_sb[:, j:j+1])
        f_pre = const.tile([U, QCHUNK * TV], f16, name="f_pre")
        nc.scalar.activation(f_pre[:, :], s_pre[:, :], AF.Tanh)
        ps_scT = [psc.tile([128, TQ], f32, tag=f"scT{h}", name=f"ps_scT{h}")
                  for h in range(NVH)]
        # pre-write scT once so 'rest' phase has data
        for h in range(NVH):
            for j in range(QCHUNK):
                nc.tensor.matmul(ps_scT[h][:, j:j+1],
                                 f_pre[:, j*TV+h*128: j*TV+(h+1)*128],
                                 vw16_sb[:, 0:1], start=True, stop=True)

        def body():
            if phase == "dve":
                for q0 in range(0, TQ, QCHUNK):
                    s_ch = spool.tile([U, QCHUNK * TV], f32, tag="s", name="s_ch")
                    for j in range(QCHUNK):
                        nc.vector.tensor_scalar_add(
                            s_ch[:, j*TV:(j+1)*TV], ka_sb[:, :],
                            qa_sb[:, q0+j:q0+j+1])
            elif phase == "act":
                for q0 in range(0, TQ, QCHUNK):
                    f_ch = fpool.tile([U, QCHUNK * TV], f16, tag="f", name="f_ch")
                    nc.scalar.activation(f_ch[:, :], s_pre[:, :], AF.Tanh)
            elif phase == "pe":
                for q0 in range(0, TQ, QCHUNK):
                    for j in range(QCHUNK):
                        q = q0 + j
                        for h in range(NVH):
                            nc.tensor.matmul(
                                ps_scT[h][:, q:q+1],
                                f_pre[:, j*TV+h*128: j*TV+(h+1)*128],
                                vw16_sb[:, 0:1], start=True, stop=True)
            elif phase == "rest":
                expT_sb = []
                for h in range(NVH):
                    e = work.tile([128, TQ], f32, tag=f"expT{h}", name=f"expT{h}")
                    nc.scalar.activation(e[:, :], ps_scT[h][:, :], AF.Exp,
                                         bias=maskc_sb[:, h:h+1])
                    expT_sb.append(e)
                ps_den = psum.tile([1, TQ], f32, tag="pb", name="ps_den")
                for h in range(NVH):
                    nc.tensor.matmul(ps_den[:, :], ones_c_sb[:, :], expT_sb[h][:, :],
                                     start=(h == 0), stop=(h == NVH - 1))
                den_sb = work.tile([1, TQ], f32, name="den_sb")
                nc.vector.tensor_copy(den_sb[:, :], ps_den[:, :])
                rinv_sb = work.tile([1, TQ], f32, name="rinv_sb")
                nc.vector.reciprocal(rinv_sb[:, :], den_sb[:, :])
                ps_rrep = psum.tile([128, TQ], f32, tag="pb", name="ps_rrep")
                nc.tensor.matmul(ps_rrep[:, :], ones_r_sb[:, :], rinv_sb[:, :],
                                 start=True, stop=True)
                rrep_sb = work.tile([128, TQ], f32, name="rrep_sb")
                nc.vector.tensor_copy(rrep_sb[:, :], ps_rrep[:, :])
                wT_sb = []
                for h in range(NVH):
                    w = work.tile([128, TQ], f32, tag=f"wT{h}", name=f"wT{h}")
                    nc.vector.tensor_mul(w[:, :], expT_sb[h][:, :], rrep_sb[:, :])
                    wT_sb.append(w)
                w_sb = work.tile([TQ, TV], f32, name="w_sb")
                for h in range(NVH):
                    ps_wn = psum.tile([128, 128], f32, tag="pb", name="ps_wn")
                    nc.tensor.transpose(ps_wn[:, :], wT_sb[h][:, :], iden_sb[:, :])
                    nc.vector.tensor_copy(w_sb[:, h*128:(h+1)*128], ps_wn[:, :])
                nc.sync.dma_start(out=out_w, in_=w_sb[:, :])
                ps_at = psum.tile([TQ, D], f32, tag="pc", name="ps_at")
                for h in range(NVH):
                    nc.tensor.matmul(ps_at[:, :], wT_sb[h][:, :], vn_sb[h][:, :],
                                     start=(h == 0), stop=(h == NVH - 1))
                x_sb = work.tile([TQ, D], f32, name="x_sb")
                nc.vector.tensor_add(x_sb[:, :], qn_sb[:, :], ps_at[:, :])
                ssum = work.tile([TQ, 1], f32, name="ssum")
                nc.vector.reduce_sum(ssum[:, :], x_sb[:, :], axis=AX.X)
                negmu = work.tile([TQ, 1], f32, name="negmu")
                nc.vector.tensor_scalar_mul(negmu[:, :], ssum[:, :], -1.0 / D)
                xc_sb = work.tile([TQ, D], f32, name="xc_sb")
                nc.vector.tensor_scalar_add(xc_sb[:, :], x_sb[:, :], negmu[:, 0:1])
                xsq = work.tile([TQ, D], f32, name="xsq")
                nc.vector.tensor_mul(xsq[:, :], xc_sb[:, :], xc_sb[:, :])
                vsum = work.tile([TQ, 1], f32, name="vsum")
                nc.vector.reduce_sum(vsum[:, :], xsq[:, :], axis=AX.X)
                veps = work.tile([TQ, 1], f32, name="veps")
                nc.vector.tensor_scalar(veps[:, :], vsum[:, :], 1.0 / D, LN_EPS,
                                        op0=ALU.mult, op1=ALU.add)
                u_t = work.tile([TQ, 1], f32, name="u_t")
                nc.vector.tensor_scalar_add(u_t[:, :], veps[:, :], 1.0)
                w_t = work.tile([TQ, 1], f32, name="w_t")
                nc.vector.reciprocal(w_t[:, :], u_t[:, :])
                y_t = work.tile([TQ, 1], f32, name="y_t0")
                nc.vector.tensor_scalar_mul(y_t[:, :], w_t[:, :], 2.0)
                e_t = work.tile([TQ, 1], f32, name="e_t")
                nc.vector.tensor_scalar_mul(e_t[:, :], veps[:, :], 0.5)
                for it in range(6):
                    a_t = work.tile([TQ, 1], f32, tag="nwa", name=f"nwa{it}")
                    nc.vector.tensor_mul(a_t[:, :], y_t[:, :], y_t[:, :])
                    b_t = work.tile([TQ, 1], f32, tag="nwb", name=f"nwb{it}")
                    nc.vector.tensor_mul(b_t[:, :], e_t[:, :], a_t[:, :])
                    c_t = work.tile([TQ, 1], f32, tag="nwc", name=f"nwc{it}")
                    nc.vector.tensor_scalar(c_t[:, :], b_t[:, :], -1.0, 1.5,
                                            op0=ALU.mult, op1=ALU.add)
                    y_new = work.tile([TQ, 1], f32, tag="nwy", name=f"nwy{it}")
                    nc.vector.tensor_mul(y_new[:, :], y_t[:, :], c_t[:, :])
                    y_t = y_new
                xn_sb = work.tile([TQ, D], f32, name="xn_sb")
                nc.vector.tensor_scalar_mul(xn_sb[:, :], xc_sb[:, :], y_t[:, 0:1])
                res_sb = work.tile([TQ, D], f32, name="res_sb")
                nc.vector.tensor_mul(res_sb[:, :], xn_sb[:, :], gam_sb[:, :])
                nc.vector.tensor_add(res_sb[:, :], res_sb[:, :], bet_sb[:, :])
                nc.sync.dma_start(out=out_res, in_=res_sb[:, :])

        if repeat:
            with tc.For_i(0, repeat, 1, hint_engines=(
                    mybir.EngineType.PE, mybir.EngineType.DVE,
                    mybir.EngineType.Activation, mybir.EngineType.SP,
                    mybir.EngineType.Pool)):
                body()
        else:
            body()

        if phase != "rest":
            nc.sync.dma_start(out=out_w[:, 0:TV], in_=ka_sb[:, :])
            nc.sync.dma_start(out=out_res[:, 0:TQ], in_=qa_sb[:, :])

    nc.compile()
    return nc



# revision 21
# speedup vs baseline: 27.7761x; 27.7761x over previous
"""Trainium2 Bass kernel for Bahdanau-style additive attention (nn_Attention).

reference math (per batch b, all fp32):
  q_attn = query @ Wq_w + Wq_b                       [B,Tq,U]
  k_attn = value @ Wk_w + Wk_b                       [B,Tv,U]
  scores[b,q,v] = sum_u V_w[u]*tanh(q_attn[b,q,u]+k_attn[b,v,u]) + V_b
  scores -= 1e9 * (~v_mask)
  weights = softmax(scores, axis=-1)
  attn = weights @ value
  result = layer_norm(query + attn) * gamma + beta
  returns (result, weights)

Sharding: data-parallel over batch B=8 -> one batch element per NeuronCore.

Key structural choices (vs a naive port):
  * v-compaction on host: masked v positions produce exactly-0 weights in
    the reference (exp(-1e9) underflows), so only the valid rows of
    `value` are shipped/computed.  TVC = max valid count rounded up to a
    multiple of 8 (136 for the seed-0 data).  Output weights are
    scattered back into the full [Tq,Tv] zeros on host.
  * all inputs ride in 3 packed blob DMAs (SP dma_start costs ~500ns of
    sequencer time each; 13 separate loads would serialize ~6.5us).
  * feats are built v-major: s[u, v*TQ+q] = qa[u,q] + ka[u,v] + (bq+bk)[u]
    via one DVE tensor_scalar per v (qa is the tensor operand, ka column
    and combined bias are the two per-partition scalars).  fp16 in/out ->
    DVE 4x mode.
  * tanh on ACT in big chunks (fp16), one instruction per v-chunk.
  * scores via per-128-column PE "matvec": lhsT = feats[u, 128 cols of
    (q,v) pairs], rhs = V_w [u,1].  Each output column of a matmul is an
    independent dot product, and with v-major packing each matmul's
    output column IS scores[all q, one v] -> scores land in PSUM already
    in natural [q, v] layout.  No transposes anywhere in the softmax.
  * padding mask pre-written with a single K=1 matmul (ones x maskrow).
  * softmax in natural layout: ACT exp with fused row-sum accumulator
    (split per 128-wide v-range so the big expT transpose + attention
    matmul overlap the last feats chunk), DVE reciprocal; attention uses
    the *unnormalized* exp and folds the 1/den scaling into a fused
    scalar_tensor_tensor residual add (which also emits the row-sum for
    the LN mean).
  * LN: ACT Square with fused accumulator for E[x^2]; var = E[x^2]-mu^2;
    rsqrt by 3 Newton iterations off a 2/(1+v) seed (ACT sqrt/rsqrt are
    broken in this runtime); gamma==1/beta==0 fast path compiled when the
    inputs allow it.
"""

import numpy as np

B, TQ, TV, D, U = 8, 128, 256, 256, 128
LN_EPS = 1e-3
N_CORES = 8
NEG_BIG = -60000.0  # padding-mask bias; exp(score + NEG_BIG) == 0 in fp32

_CACHE = {}


def _offsets(tvc):
    """Column offsets inside the four input blobs."""
    o16 = {}
    c = 0
    for name, w in (("wq0", 128), ("wq1", 128), ("qT0", TQ), ("qT1", TQ)):
        o16[name] = (c, w)
        c += w
    n16 = c
    o16b = {}
    c = 0
    for name, w in (("wk0", 128), ("wk1", 128), ("vT0", tvc), ("vT1", tvc)):
        o16b[name] = (c, w)
        c += w
    n16b = c
    o2 = {}
    c = 0
    for name, w in (("vcA", D), ("vw", 1), ("maskr", tvc), ("ones", TQ),
                    ("vcB", D)):
        o2[name] = (c, w)
        c += w
    n2 = c
    o32 = {}
    c = 0
    for name, w in (("qn", D), ("iden", 128), ("bqk", 1)):
        o32[name] = (c, w)
        c += w
    n32 = c
    return o16, n16, o16b, n16b, o2, n2, o32, n32


def _chunks(tvc):
    """Feats v-chunks of <=16, aligned to the 128 boundary.

    The >=128 tail range goes FIRST so its exp/transpose/attention matmul
    run early; the [0,128) range is ramped (8-wide first chunk so ACT can
    start sooner) and ends with a small chunk to shorten the final
    exp-critical path.
    """
    out = []
    v0 = 128
    while v0 < tvc:
        ch = min(16, tvc - v0)
        out.append((v0, ch))
        v0 += ch
    lim = min(tvc, 128)
    v0 = 0
    plan = (8, 16, 16, 16, 16, 16, 16, 16, 8)
    pi = 0
    while v0 < lim:
        ch = min(plan[pi] if pi < len(plan) else 16, lim - v0)
        out.append((v0, ch))
        v0 += ch
        pi += 1
    return out


def _build_program(tvc, trivial_ln=True, repeat=0):
    from contextlib import ExitStack
    import concourse.bacc as bacc
    import concourse.tile as tile
    from concourse import mybir

    f32 = mybir.dt.float32
    f16 = mybir.dt.float16
    AF = mybir.ActivationFunctionType
    ALU = mybir.AluOpType

    nc = bacc.Bacc("TRN2", target_bir_lowering=False, debug=False)

    o16, n16, o16b, n16b, o2, n2, o32, n32 = _offsets(tvc)
    blob1 = nc.dram_tensor("blob1", [128, n16], f16, kind="ExternalInput").ap()
    blob1b = nc.dram_tensor("blob1b", [128, n16b], f16, kind="ExternalInput").ap()
    blob2 = nc.dram_tensor("blob2", [128, n2], f16, kind="ExternalInput").ap()
    blob3 = nc.dram_tensor("blob3", [128, n32], f32, kind="ExternalInput").ap()
    if not trivial_ln:
        gamd = nc.dram_tensor("gam", [TQ, D], f32, kind="ExternalInput").ap()
        betd = nc.dram_tensor("bet", [TQ, D], f32, kind="ExternalInput").ap()

    out_res = nc.dram_tensor("out_res", [TQ, D], f32, kind="ExternalOutput").ap()
    out_w = nc.dram_tensor("out_w", [TQ, tvc], f32, kind="ExternalOutput").ap()

    chunks = _chunks(tvc)
    # v-partition ranges for exp/transpose/attention; the >=128 tail first,
    # matching the feats chunk order, so the big [0,128) range (done last)
    # is the only thing on the final critical path.
    vch = [(i, min(128, tvc - i)) for i in range(0, tvc, 128)][::-1]

    with tile.TileContext(nc) as tc, ExitStack() as ctx:
        const = ctx.enter_context(tc.tile_pool(name="const", bufs=1))
        work = ctx.enter_context(tc.tile_pool(name="work", bufs=2))
        spool = ctx.enter_context(tc.tile_pool(name="spool", bufs=2))
        fpool = ctx.enter_context(tc.tile_pool(name="fpool", bufs=2))
        psA = ctx.enter_context(tc.tile_pool(name="psA", bufs=1, space="PSUM"))
        psB = ctx.enter_context(tc.tile_pool(name="psB", bufs=1, space="PSUM"))
        psC = ctx.enter_context(tc.tile_pool(name="psC", bufs=1, space="PSUM"))
        psD = ctx.enter_context(tc.tile_pool(name="psD", bufs=2, space="PSUM"))
        psE = ctx.enter_context(tc.tile_pool(name="psE", bufs=1, space="PSUM"))

        def body():
            b1 = const.tile([128, n16], f16, name="b1")
            nc.sync.dma_start(out=b1[:, :], in_=blob1)
            b1b = const.tile([128, n16b], f16, name="b1b")
            nc.sync.dma_start(out=b1b[:, :], in_=blob1b)
            b2 = const.tile([128, n2], f16, name="b2")
            nc.sync.dma_start(out=b2[:, :], in_=blob2)
            b3 = const.tile([128, n32], f32, name="b3")
            nc.sync.dma_start(out=b3[:, :], in_=blob3)

            def s16(name, rows=128):
                if name in o16:
                    c, w = o16[name]
                    return b1[0:rows, c:c + w]
                c, w = o16b[name]
                return b1b[0:rows, c:c + w]

            def s2(name, r0=0, rows=128):
                c, w = o2[name]
                return b2[r0:r0 + rows, c:c + w]

            def s32(name, rows=128):
                c, w = o32[name]
                return b3[0:rows, c:c + w]

            if not trivial_ln:
                gam_sb = const.tile([TQ, D], f32, name="gam_sb")
                nc.sync.dma_start(out=gam_sb[:, :], in_=gamd)
                bet_sb = const.tile([TQ, D], f32, name="bet_sb")
                nc.sync.dma_start(out=bet_sb[:, :], in_=betd)

            # ---- q_attn^T [u,q], k_attn^T [u,v] (biases folded later) ----
            ps_qa = psA.tile([U, TQ], f32, tag="qa")
            nc.tensor.matmul(ps_qa[:, :], s16("wq0"), s16("qT0"),
                             start=True, stop=False)
            nc.tensor.matmul(ps_qa[:, :], s16("wq1"), s16("qT1"),
                             start=False, stop=True)
            ps_ka = psB.tile([U, tvc], f32, tag="ka")
            nc.tensor.matmul(ps_ka[:, :], s16("wk0"), s16("vT0"),
                             start=True, stop=False)
            nc.tensor.matmul(ps_ka[:, :], s16("wk1"), s16("vT1"),
                             start=False, stop=True)

            qa_sb = work.tile([U, TQ], f16, name="qa_sb")
            nc.vector.tensor_copy(qa_sb[:, :], ps_qa[:, :])
            # ka stays in PSUM: the per-v scalar operand of the s-build
            # reads ps_ka columns directly (scalar APs are exempt from the
            # DVE 4x-mode operand checks), so no ka copy is needed
            ka_sb = ps_ka

            # ---- feats pipeline: s = qa + ka_v + bqk; tanh; matvec ----
            # one PSUM bank per v-range so each range's accumulation group
            # can be closed (and exp'd) independently
            sc_tiles = {}
            last_v = {}
            for (i0, n) in vch:
                t = psC.tile([TQ, n], f32, tag=f"sc{i0}", name=f"sc{i0}")
                # padding mask first: scores[:, v] = maskr[v] (rank-1);
                # columns then accumulate on top with start=False
                c0, _ = o2["maskr"]
                nc.tensor.matmul(t[:, :], s2("ones", rows=1),
                                 b2[0:1, c0 + i0:c0 + i0 + n],
                                 start=True, stop=False)
                sc_tiles[i0] = t
            for (v0, ch) in chunks:
                for (i0, n) in vch:
                    if i0 <= v0 < i0 + n:
                        last_v[i0] = v0 + ch - 1
            # ---- feats chunks + per-v-range softmax head, interleaved so
            # the tail range's exp/transpose run as soon as its chunks are
            # done (engine queues are FIFO in emission order) ----
            exp_sb = work.tile([TQ, tvc], f32, name="exp_sb")
            dens, et_sb = [], []

            pending_copies = []

            def emit_softmax_head(i, i0, n):
                den = work.tile([TQ, 1], f32, tag=f"den{i}", name=f"den{i}")
                nc.scalar.activation(exp_sb[:, i0:i0 + n],
                                     sc_tiles[i0][:, :], AF.Exp,
                                     accum_out=den[:, 0:1])
                dens.append(den)
                ps_t = psD.tile([128, 128], f32, tag="tr", name=f"tr{i0}")
                nc.tensor.transpose(ps_t[0:n, 0:TQ], exp_sb[:, i0:i0 + n],
                                    s32("iden"))
                # the DVE fp16 cast is deferred a chunk group so it never
                # head-of-line blocks the s-build stream behind it
                et = work.tile([n, TQ], f16, tag=f"et{i}", name=f"et{i}")
                et_sb.append(et)
                pending_copies.append((et, ps_t, n))

            def flush_copies():
                while pending_copies:
                    et, ps_t, n = pending_copies.pop(0)
                    nc.vector.tensor_copy(et[:, :], ps_t[0:n, 0:TQ])

            done_ranges = 0
            covered = [0] * len(vch)
            for ci, (v0, ch) in enumerate(chunks):
                flush_copies()
                s_ch = spool.tile([U, 32 * TQ], f16, tag="s", name="s_ch")
                for j in range(ch):
                    nc.vector.tensor_scalar(
                        s_ch[:, j * TQ:(j + 1) * TQ], qa_sb[:, :],
                        ka_sb[:, v0 + j:v0 + j + 1], s32("bqk"),
                        op0=ALU.add, op1=ALU.add)
                f_ch = fpool.tile([U, 32 * TQ], f16, tag="f", name="f_ch")
                nc.scalar.activation(f_ch[:, 0:ch * TQ], s_ch[:, 0:ch * TQ],
                                     AF.Tanh)
                ri0 = next(i0 for (i0, n) in vch if i0 <= v0 < i0 + n)
                sct = sc_tiles[ri0]
                for j in range(ch):
                    nc.tensor.matmul(
                        sct[:, v0 - ri0 + j:v0 - ri0 + j + 1],
                        f_ch[:, j * TQ:(j + 1) * TQ], s2("vw"),
                        start=False, stop=(v0 + j == last_v[ri0]))
                # when a v-range is fully covered, emit its softmax head now
                for i, (i0, n) in enumerate(vch):
                    if i0 <= v0 < i0 + n:
                        covered[i] += ch
                        if covered[i] == n:
                            emit_softmax_head(i, i0, n)
                            done_ranges += 1
            assert done_ranges == len(vch)
            if len(dens) > 1:
                den_all = work.tile([TQ, 1], f32, name="den_all")
                nc.vector.tensor_add(den_all[:, :], dens[0][:, :],
                                     dens[1][:, :])
            else:
                den_all = dens[0]
            rinv_sb = work.tile([TQ, 1], f32, name="rinv_sb")
            nc.vector.reciprocal(rinv_sb[:, :], den_all[:, :])

            # normalized weights -> DRAM (off critical path)
            w_sb = work.tile([TQ, tvc], f32, name="w_sb")
            nc.vector.tensor_scalar_mul(w_sb[:, :], exp_sb[:, :],
                                        rinv_sb[:, 0:1])
            nc.sync.dma_start(out=out_w, in_=w_sb[:, :])

            # ---- attention with unnormalized exp ----
            ps_at = psE.tile([TQ, D], f32, tag="at")
            for i, (i0, n) in enumerate(vch):
                vc_ap = s2("vcA") if i0 == 0 else s2("vcB", rows=n)
                nc.tensor.matmul(ps_at[:, :], et_sb[i][:, :], vc_ap,
                                 start=(i == 0), stop=(i == len(vch) - 1))

            # ---- residual + layernorm ----
            # x = attn*rinv + qn ; ssum = rowsum(x)
            x_sb = work.tile([TQ, D], f32, name="x_sb")
            ssum = work.tile([TQ, 1], f32, name="ssum")
            nc.vector.scalar_tensor_tensor(x_sb[:, :], ps_at[:, :],
                                           rinv_sb[:, 0:1], s32("qn"),
                                           op0=ALU.mult, op1=ALU.add,
                                           accum_out=ssum[:, 0:1])
            negmu = work.tile([TQ, 1], f32, name="negmu")
            nc.vector.tensor_scalar_mul(negmu[:, :], ssum[:, :], -1.0 / D)
            # E[x^2] via ACT square with accumulator
            xsq = work.tile([TQ, D], f16, name="xsq")
            sqs = work.tile([TQ, 1], f32, name="sqs")
            nc.scalar.activation(xsq[:, :], x_sb[:, :], AF.Square,
                                 accum_out=sqs[:, 0:1])
            # var = E[x^2] - mu^2 ; veps = var + eps
            mu_sb = work.tile([TQ, 1], f32, name="mu_sb")
            nc.vector.tensor_scalar_mul(mu_sb[:, :], ssum[:, :], 1.0 / D)
            m2_sb = work.tile([TQ, 1], f32, name="m2_sb")
            nc.vector.tensor_mul(m2_sb[:, :], mu_sb[:, :], mu_sb[:, :])
            ve0 = work.tile([TQ, 1], f32, name="ve0")
            nc.vector.tensor_scalar(ve0[:, :], sqs[:, :], 1.0 / D, LN_EPS,
                                    op0=ALU.mult, op1=ALU.add)
            veps = work.tile([TQ, 1], f32, name="veps")
            nc.vector.tensor_sub(veps[:, :], ve0[:, :], m2_sb[:, :])
            # rsqrt: y0 = 2/(1+v), 3 Newton iterations
            u_t = work.tile([TQ, 1], f32, name="u_t")
            nc.vector.tensor_scalar_add(u_t[:, :], veps[:, :], 1.0)
            w_t = work.tile([TQ, 1], f32, name="w_t")
            nc.vector.reciprocal(w_t[:, :], u_t[:, :])
            y_t = work.tile([TQ, 1], f32, name="y0")
            nc.vector.tensor_scalar_mul(y_t[:, :], w_t[:, :], 2.0)
            e_t = work.tile([TQ, 1], f32, name="e_t")
            nc.vector.tensor_scalar_mul(e_t[:, :], veps[:, :], 0.5)
            c15 = work.tile([TQ, 1], f32, name="c15")
            nc.vector.tensor_scalar(c15[:, :], e_t[:, :], 0.0, 1.5,
                                    op0=ALU.mult, op1=ALU.add)
            for it in range(2):
                b_t = work.tile([TQ, 1], f32, tag="nwb", name=f"nwb{it}")
                nc.vector.scalar_tensor_tensor(b_t[:, :], y_t[:, :],
                                               y_t[:, 0:1], e_t[:, :],
                                               op0=ALU.mult, op1=ALU.mult)
                t_t = work.tile([TQ, 1], f32, tag="nwt", name=f"nwt{it}")
                nc.vector.scalar_tensor_tensor(t_t[:, :], b_t[:, :], -1.0,
                                               c15[:, :],
                                               op0=ALU.mult, op1=ALU.add)
                y_n = work.tile([TQ, 1], f32, tag="nwy", name=f"nwy{it}")
                nc.vector.tensor_mul(y_n[:, :], y_t[:, :], t_t[:, :])
                y_t = y_n
            # result = (x - mu) * rstd [* gamma + beta]
            res_sb = work.tile([TQ, D], f32, name="res_sb")
            nc.vector.tensor_scalar(res_sb[:, :], x_sb[:, :], negmu[:, 0:1],
                                    y_t[:, 0:1], op0=ALU.add, op1=ALU.mult)
            if not trivial_ln:
                r2 = work.tile([TQ, D], f32, name="r2")
                nc.vector.tensor_mul(r2[:, :], res_sb[:, :], gam_sb[:, :])
                nc.vector.tensor_add(r2[:, :], r2[:, :], bet_sb[:, :])
                res_sb = r2
            nc.sync.dma_start(out=out_res, in_=res_sb[:, :])

        if repeat:
            with tc.For_i(0, repeat, 1, hint_engines=(
                    mybir.EngineType.PE, mybir.EngineType.DVE,
                    mybir.EngineType.Activation, mybir.EngineType.SP)):
                body()
        else:
            body()

    nc.compile()
    return nc


def _plan(v_mask):
    counts = v_mask.sum(axis=1)
    tvc = int(-(-max(int(counts.max()), 8) // 8) * 8)
    idxs = [np.where(v_mask[b])[0] for b in range(v_mask.shape[0])]
    return tvc, idxs


def _host_prep(query, value, v_mask, Wq_w, Wq_b, Wk_w, Wk_b, V_w, ln_gamma,
               ln_beta, tvc, idxs, trivial_ln):
    f16 = np.float16
    f32 = np.float32
    o16, n16, o16b, n16b, o2, n2, o32, n32 = _offsets(tvc)

    def put(blob, off, arr):
        c, w = off
        r, w2 = arr.shape
        blob[:r, c:c + w2] = arr

    wq16 = Wq_w.astype(f16)
    wk16 = Wk_w.astype(f16)
    in_maps = []
    for b in range(B):
        q = query[b].astype(f32)
        idx = idxs[b]
        cnt = len(idx)
        vcomp = np.zeros((tvc, D), f32)
        vcomp[:cnt] = value[b][idx]
        vT = vcomp.T.astype(f16)
        qT = q.T.astype(f16)

        b1 = np.zeros((128, n16), f16)
        put(b1, o16["wq0"], wq16[0:128])
        put(b1, o16["wq1"], wq16[128:256])
        put(b1, o16["qT0"], qT[0:128])
        put(b1, o16["qT1"], qT[128:256])
        b1b = np.zeros((128, n16b), f16)
        put(b1b, o16b["wk0"], wk16[0:128])
        put(b1b, o16b["wk1"], wk16[128:256])
        put(b1b, o16b["vT0"], vT[0:128])
        put(b1b, o16b["vT1"], vT[128:256])

        b2 = np.zeros((128, n2), f16)
        put(b2, o2["vcA"], vcomp[0:128].astype(f16))
        if tvc > 128:
            put(b2, o2["vcB"], vcomp[128:tvc].astype(f16))
        put(b2, o2["vw"], V_w.astype(f16).reshape(U, 1))
        maskr = np.full((1, tvc), NEG_BIG, f16)
        maskr[0, :cnt] = 0.0
        put(b2, o2["maskr"], maskr)
        put(b2, o2["ones"], np.ones((1, TQ), f16))

        b3 = np.zeros((128, n32), f32)
        put(b3, o32["qn"], q)
        put(b3, o32["iden"], np.eye(128, dtype=f32))
        put(b3, o32["bqk"], (Wq_b.astype(f32) + Wk_b.astype(f32)).reshape(U, 1))

        m = {"blob1": b1, "blob1b": b1b, "blob2": b2, "blob3": b3}
        if not trivial_ln:
            m["gam"] = np.broadcast_to(ln_gamma.astype(f32), (TQ, D)).copy()
            m["bet"] = np.broadcast_to(ln_beta.astype(f32), (TQ, D)).copy()
        in_maps.append(m)
    return in_maps


def kernel(query, value, v_mask, Wq_w, Wq_b, Wk_w, Wk_b, V_w, V_b, ln_gamma,
           ln_beta):
    from concourse.bass_utils import run_bass_kernel_spmd

    query = np.asarray(query, np.float32)
    value = np.asarray(value, np.float32)
    v_mask = np.asarray(v_mask, bool)
    tvc, idxs = _plan(v_mask)
    trivial_ln = bool(np.all(np.asarray(ln_gamma) == 1.0)
                      and np.all(np.asarray(ln_beta) == 0.0))
    key = (tvc, trivial_ln)
    if key not in _CACHE:
        _CACHE[key] = _build_program(tvc, trivial_ln)
    nc = _CACHE[key]
    in_maps = _host_prep(query, value, v_mask, Wq_w, Wq_b, Wk_w, Wk_b, V_w,
                         ln_gamma, ln_beta, tvc, idxs, trivial_ln)
    res = run_bass_kernel_spmd(nc, in_maps, core_ids=list(range(N_CORES)))
    result = np.stack([res.results[b]["out_res"] for b in range(B)])
    weights = np.zeros((B, TQ, TV), np.float32)
    for b in range(B):
        cnt = len(idxs[b])
        weights[b][:, idxs[b]] = res.results[b]["out_w"][:, :cnt]
    return result.astype(np.float32), weights
